# revision 1
# baseline (speedup 1.0000x reference)
"""CMC@k accuracy kernel for Trainium2 (8 NeuronCores, SPMD).

Algorithm (per flank of G=8192 rows, D=256, k=5):
  reference = mean over rows of [any of the k nearest neighbours (excl. self)
  shares the row's label].

Reformulation that avoids argsort: for row i let
    score[i,j] = sq[j] - 2*dot[i,j]        (= dist[i,j] - sq[i], same ordering)
    dm[i]      = min over same-label j!=i of score[i,j]
    cnt[i]     = #{ j : score[i,j] < dm[i] }   (includes self, strict <)
  match[i] <=> 1 <= cnt[i] <= k.
If the row's label is unique, dm is huge and cnt=G > k -> no match, matching
the reference.

Host-side marshalling: each flank is sorted by label (the metric is
permutation invariant), so same-label points are contiguous and the masked
min only needs a narrow column window around the diagonal.  Each of the 4
cores per flank gets the sorted flank *rotated* so its own 2048 query rows
sit at local rows 0..2047 — keeping the SPMD program identical across cores;
the wrapped label-run at the rotation cut is handled by an extra window
segment at the array tail for slab 0.

Precision/perf: fp32 matmuls run at 4 cycles/row on TRN2 (2 HW passes).
Instead we split e = h + l into two fp16 halves (Dekker split, ~21-bit
combined mantissa) and compute dot = h.h' + h.l' + l.h' with six
single-pass fp16 matmuls per 512-column chunk (l.l' ~ 2^-22 dropped).
The -0.5*sq[j] term rides inside the half-0 h.l' matmul: rows 0,1 of the
l-database are replaced by the fp16 split of -0.5*sq[j] and the query-side
stationary operand has those rows set to 1.0 (the two dropped h*l terms are
~5e-4, far below the ~1.0 distance gaps that decide CMC matches).

Device per slab of 128 query rows:
  PE:  psum = h.h' + h.l'(+sq rows) + l.h'  over both 128-dim halves
  ACT: score = -2 * psum  (PSUM->SBUF, func=Copy scale=-2)
  DVE: neBIG = (lab_win != lab_i) * 1e6 (+1e6 on the self diagonal)
       dm    = min(score_win + neBIG)        (tensor_tensor + reduce-min)
       cnt   = sum(score < dm)               (tensor_scalar accum, in place)
       match = (cnt <= k)
Final: per-core match count -> [1,1] output; host sums and divides by N.
"""
import os
import sys
import numpy as np

sys.path.insert(0, "/opt/trn_rl_repo")

NUM_FLANKS = 2
N, D = 16384, 256
G = N // NUM_FLANKS            # 8192 rows per flank
NCORES = 8
CORES_PER_FLANK = NCORES // NUM_FLANKS
Q = G // CORES_PER_FLANK       # 2048 query rows per core
NSLABS = Q // 128              # 16 slabs per core
M = 64                         # window margin (>= max same-label run)
W = 128 + 2 * M                # window width
BIG = 1.0e6
CHUNK = 512                    # matmul free dim (one PSUM bank, fp32 out)
PTILE = 2048                   # evacuation granularity (4 PSUM banks)

_cached = {}


def _build_program(k: int):
    import concourse.bacc as bacc
    import concourse.tile as tile
    from concourse import mybir

    f32 = mybir.dt.float32
    f16 = mybir.dt.float16
    Alu = mybir.AluOpType
    Act = mybir.ActivationFunctionType

    nc = bacc.Bacc()
    h0_d = nc.dram_tensor("h0", [128, G], f16, kind="ExternalInput")
    h1_d = nc.dram_tensor("h1", [128, G], f16, kind="ExternalInput")
    l0_d = nc.dram_tensor("l0", [128, G], f16, kind="ExternalInput")
    l1_d = nc.dram_tensor("l1", [128, G], f16, kind="ExternalInput")
    hmod_d = nc.dram_tensor("hmod", [128, Q], f16, kind="ExternalInput")
    l0q_d = nc.dram_tensor("l0q", [128, Q], f16, kind="ExternalInput")
    labf_d = nc.dram_tensor("labf", [G], f32, kind="ExternalInput")
    diag_d = nc.dram_tensor("diag", [128, 128], f32, kind="ExternalInput")
    out_d = nc.dram_tensor("out", [1, 1], f32, kind="ExternalOutput")

    with tile.TileContext(nc) as tc:
        with tc.tile_pool(name="singles", bufs=1) as singles:
            # ---------------- load database + constants ----------------
            h0 = singles.tile([128, G], f16)
            h1 = singles.tile([128, G], f16)
            l0 = singles.tile([128, G], f16)
            l1 = singles.tile([128, G], f16)
            hmod = singles.tile([128, Q], f16)
            l0q = singles.tile([128, Q], f16)
            diag_big = singles.tile([128, 128], f32)
            nc.sync.dma_start(h0[:], h0_d[:])
            nc.sync.dma_start(h1[:], h1_d[:])
            nc.sync.dma_start(l0[:], l0_d[:])
            nc.sync.dma_start(l1[:], l1_d[:])
            nc.sync.dma_start(hmod[:], hmod_d[:])
            nc.sync.dma_start(l0q[:], l0q_d[:])
            nc.sync.dma_start(diag_big[:], diag_d[:])

            # labb: labels broadcast over partitions; layout:
            #   cols [0,M)       <- labf[G-M:G]   (wrapped tail)
            #   cols [M, M+Q+M)  <- labf[0:Q+M]
            labb = singles.tile([128, 2 * M + Q], f32)
            nc.gpsimd.dma_start(
                labb[:, 0:M], labf_d[G - M:G].partition_broadcast(128)
            )
            nc.gpsimd.dma_start(
                labb[:, M:], labf_d[0:Q + M].partition_broadcast(128)
            )
            # labiT[i, t] = labf[128 t + i]  (per-slab query labels)
            labiT = singles.tile([128, NSLABS], f32)
            nc.gpsimd.dma_start(
                labiT[:], labf_d[0:Q].rearrange("(t p) -> p t", p=128)
            )

            ones_col = singles.tile([128, 1], f32)
            nc.vector.memset(ones_col[:], 1.0)
            match_acc = singles.tile([128, NSLABS], f32)

            # ---------------- main loop over 16 slabs ----------------
            with (
                tc.tile_pool(name="scores", bufs=2) as scores,
                tc.tile_pool(name="small", bufs=2) as small,
                tc.tile_pool(name="mm", bufs=2, space="PSUM") as mmp,
            ):
                for t in range(NSLABS):
                    score = scores.tile([128, G], f32, tag="score")
                    sl = slice(128 * t, 128 * (t + 1))
                    for q in range(G // PTILE):
                        pm = mmp.tile([128, PTILE], f32, tag="mm")
                        for c in range(PTILE // CHUNK):
                            ps = pm[:, CHUNK * c:CHUNK * (c + 1)]
                            cols = slice(
                                PTILE * q + CHUNK * c, PTILE * q + CHUNK * (c + 1)
                            )
                            nc.tensor.matmul(
                                ps, h0[:, sl], h0[:, cols], start=True, stop=False
                            )
                            nc.tensor.matmul(
                                ps, hmod[:, sl], l0[:, cols], start=False, stop=False
                            )
                            nc.tensor.matmul(
                                ps, l0q[:, sl], h0[:, cols], start=False, stop=False
                            )
                            nc.tensor.matmul(
                                ps, h1[:, sl], h1[:, cols], start=False, stop=False
                            )
                            nc.tensor.matmul(
                                ps, h1[:, sl], l1[:, cols], start=False, stop=False
                            )
                            nc.tensor.matmul(
                                ps, l1[:, sl], h1[:, cols], start=False, stop=True
                            )
                        nc.scalar.activation(
                            score[:, PTILE * q:PTILE * (q + 1)],
                            pm[:],
                            Act.Copy,
                            scale=-2.0,
                        )

                    # ---- windowed masked min -> dm ----
                    lab_i = labiT[:, t:t + 1]
                    dm = small.tile([128, 1], f32, tag="dm")
                    ne = small.tile([128, W], f32, tag="ne")
                    nc.vector.tensor_scalar(
                        ne[:], labb[:, 128 * t:128 * t + W], lab_i, BIG,
                        op0=Alu.not_equal, op1=Alu.mult,
                    )
                    nc.vector.tensor_tensor(
                        out=ne[:, M:M + 128], in0=ne[:, M:M + 128],
                        in1=diag_big[:], op=Alu.add,
                    )
                    mw = small.tile([128, W], f32, tag="mw")
                    if t == 0:
                        # wrapped tail: score cols [G-M, G) sit at labb[:, 0:M]
                        nc.vector.tensor_tensor(
                            out=mw[:, 0:M], in0=score[:, G - M:G],
                            in1=ne[:, 0:M], op=Alu.add,
                        )
                        nc.vector.tensor_tensor(
                            out=mw[:, M:W], in0=score[:, 0:128 + M],
                            in1=ne[:, M:W], op=Alu.add,
                        )
                    else:
                        lo = 128 * t - M
                        nc.vector.tensor_tensor(
                            out=mw[:], in0=score[:, lo:lo + W], in1=ne[:],
                            op=Alu.add,
                        )
                    nc.vector.tensor_reduce(
                        dm[:], mw[:], axis=mybir.AxisListType.X, op=Alu.min
                    )

                    # ---- count strictly-smaller scores (in place) ----
                    cnt = small.tile([128, 1], f32, tag="cnt")
                    nc.vector.tensor_scalar(
                        score[:], score[:], dm[:], None,
                        op0=Alu.is_lt, op1=Alu.add, accum_out=cnt[:],
                    )
                    nc.vector.tensor_scalar(
                        match_acc[:, t:t + 1], cnt[:], float(k), None,
                        op0=Alu.is_le,
                    )

            # ---------------- final reduction ----------------
            msum = singles.tile([128, 1], f32)
            nc.vector.reduce_sum(msum[:], match_acc[:], axis=mybir.AxisListType.X)
            with tc.tile_pool(name="fin", bufs=1, space="PSUM") as finp:
                pf = finp.tile([1, 1], f32)
                nc.tensor.matmul(pf[:], ones_col[:], msum[:], start=True, stop=True)
                osb = singles.tile([1, 1], f32)
                nc.scalar.activation(osb[:], pf[:], Act.Copy)
                nc.sync.dma_start(out_d[:], osb[:])

    nc.finalize()
    return nc


def _prepare_inputs(embeddings, labels):
    """Sort each flank by label, build per-core rotated fp16 split inputs."""
    emb = np.ascontiguousarray(np.asarray(embeddings, dtype=np.float32))
    lab = np.asarray(labels)
    diag = (np.eye(128) * BIG).astype(np.float32)
    in_maps = []
    for f in range(NUM_FLANKS):
        ef = emb[f * G:(f + 1) * G]
        lf = lab[f * G:(f + 1) * G]
        order = np.argsort(lf, kind="stable")
        ef, lf = ef[order], lf[order]
        # window-margin safety: same-label runs must fit in M
        runs = np.diff(
            np.flatnonzero(np.concatenate(([True], lf[1:] != lf[:-1], [True])))
        )
        assert runs.max() <= M, f"label run {runs.max()} exceeds window margin {M}"
        lf32 = lf.astype(np.float32)
        for cc in range(CORES_PER_FLANK):
            r = Q * cc
            e = np.ascontiguousarray(np.roll(ef, -r, axis=0))
            h = e.astype(np.float16)
            low = (e - h.astype(np.float32)).astype(np.float16)
            hT = np.ascontiguousarray(h.T)           # [256, G]
            lT = np.ascontiguousarray(low.T)
            sqb = -0.5 * np.einsum(
                "ij,ij->i", e.astype(np.float64), e.astype(np.float64)
            ).astype(np.float32)
            sh = sqb.astype(np.float16)
            slo = (sqb - sh.astype(np.float32)).astype(np.float16)
            l0 = lT[0:128].copy()
            l0q = np.ascontiguousarray(l0[:, 0:Q])   # true query lows, half 0
            l0[0, :] = sh                            # -0.5*sq rides rows 0,1
            l0[1, :] = slo
            hmod = np.ascontiguousarray(hT[0:128, 0:Q])
            hmod[0:2, :] = np.float16(1.0)
            in_maps.append({
                "h0": np.ascontiguousarray(hT[0:128]),
                "h1": np.ascontiguousarray(hT[128:256]),
                "l0": l0,
                "l1": np.ascontiguousarray(lT[128:256]),
                "hmod": hmod,
                "l0q": l0q,
                "labf": np.ascontiguousarray(np.roll(lf32, -r)),
                "diag": diag,
            })
    return in_maps


def kernel(embeddings, labels, flanks, k):
    from concourse.bass_utils import run_bass_kernel_spmd

    k = int(k)
    if ("nc", k) not in _cached:
        _cached[("nc", k)] = _build_program(k)
    nc = _cached[("nc", k)]
    in_maps = _prepare_inputs(embeddings, labels)
    res = run_bass_kernel_spmd(nc, in_maps, list(range(NCORES)))
    total = sum(float(r["out"][0, 0]) for r in res.results)
    return np.float32(total / N)


if __name__ == "__main__":
    sys.path.insert(0, os.path.dirname(os.path.abspath(__file__)))
    from reference import setup_inputs, reference

    inputs = setup_inputs()
    expected = float(reference(**inputs))
    got = float(kernel(**{kk: np.asarray(v) for kk, v in inputs.items()}))
    rel = abs(got - expected) / abs(expected)
    print(f"expected={expected} got={got} rel={rel:.3e}")



# revision 4
# speedup vs baseline: 1.8031x; 1.8031x over previous
"""CMC@k accuracy kernel for Trainium2 (8 NeuronCores, SPMD).

Algorithm (per flank of G=8192 rows, D=256, k=5):
  reference = mean over rows of [any of the k nearest neighbours (excl. self)
  shares the row's label].

Reformulation that avoids argsort: for row i let
    score[i,j] = sq[j] - 2*dot[i,j]        (= dist[i,j] - sq[i], same ordering)
    dm[i]      = min over same-label j!=i of score[i,j]
    ssum[i]    = sum_j sign(score[i,j] - dm[i])   (= #greater - #less)
  match[i] <=> cnt <= k where cnt = #less = (G - ties - ssum)/2, i.e.
  match[i] <=> ssum >= G - 1 - 2k   (ties == 1: the defining neighbour).

Host-side marshalling: each flank is sorted by label (the metric is
permutation invariant), so same-label points are contiguous and the masked
min only needs a narrow 256-column window around the diagonal.  Each of the
4 cores per flank gets the sorted flank rotated so its own 2048 query rows
sit at db columns 64..2112 (extra +64 roll => the label window for query
slab t is exactly db cols [128t, 128t+256) -- never wraps).

Precision: fp32 matmuls are slow on PE (4 passes); instead e is split
e = h + l into fp16 high/low halves and score is built from THREE
single-pass fp16 matmuls per 512-column chunk:
    (-2 h_q0).h_db0  +  hqm.l0  +  (-2 h_q1).h_db1
where l0 rows 0,1 are replaced by the fp16 split of +||e_j||^2 and hqm is
-2 h_q0 with rows 0,1 set to 1.0 (so PSUM = sq[j] - 2*dot directly; the -2
prescale of the stationary operands is exact in fp16).  The dropped
low-order terms (query-low x db, and db-low of half 1) shift scores by
~8e-3 which was verified on the reference inputs to flip zero match
decisions (margins between k-th/k+1-th neighbour distances are O(1)).

Device per slab of 128 query rows:
  PE:  psum = score directly (3 fp16 matmuls per 512-col chunk)
  DVE: mw   = -psum_win + nen  (nen = -(BIG*(label!=) + BIG*diag), host-made)
       mdm  = max(mw) = -dm
  ACT: sgn  = Sign(psum + mdm), accum_out = per-ptile sign-sum
       (evacuation and counting fused in one pass over PSUM)
  DVE: match = (sum of sign-sums >= G-1-2k)
Final: per-core match count -> [1,1] output; host sums and divides by N.
"""
import os
import sys
import numpy as np

sys.path.insert(0, "/opt/trn_rl_repo")

NUM_FLANKS = 2
N, D = 16384, 256
G = N // NUM_FLANKS            # 8192 rows per flank
NCORES = 8
CORES_PER_FLANK = NCORES // NUM_FLANKS
Q = G // CORES_PER_FLANK       # 2048 query rows per core
NSLABS = Q // 128              # 16 slabs per core
M = 64                         # window margin (>= max same-label run)
W = 256                        # window width
ROLL = 64                      # extra db roll so windows never wrap
BIG = 1.0e6
CHUNK = 512                    # matmul free dim (fp32 out, one PSUM bank)
PTILE = 2048                   # psum tile (4 banks)

_cached = {}


def _build_program(k: int):
    import concourse.bacc as bacc
    import concourse.tile as tile
    from concourse import mybir

    f32 = mybir.dt.float32
    f16 = mybir.dt.float16
    Alu = mybir.AluOpType
    Act = mybir.ActivationFunctionType

    nc = bacc.Bacc()
    h0_d = nc.dram_tensor("h0", [128, G], f16, kind="ExternalInput")
    h1_d = nc.dram_tensor("h1", [128, G], f16, kind="ExternalInput")
    l0_d = nc.dram_tensor("l0", [128, G], f16, kind="ExternalInput")
    hq0_d = nc.dram_tensor("hq0", [128, Q], f16, kind="ExternalInput")
    hqm_d = nc.dram_tensor("hqm", [128, Q], f16, kind="ExternalInput")
    hq1_d = nc.dram_tensor("hq1", [128, Q], f16, kind="ExternalInput")
    nen_d = nc.dram_tensor("nen", [128, NSLABS * W], f32, kind="ExternalInput")
    out_d = nc.dram_tensor("out", [1, 1], f32, kind="ExternalOutput")

    thresh = float(G - 1 - 2 * k)

    with tile.TileContext(nc) as tc:
        with tc.tile_pool(name="singles", bufs=1) as singles:
            # ---------------- load inputs (priority order) ----------------
            hq0 = singles.tile([128, Q], f16)
            hqm = singles.tile([128, Q], f16)
            hq1 = singles.tile([128, Q], f16)
            h0 = singles.tile([128, G], f16)
            h1 = singles.tile([128, G], f16)
            l0 = singles.tile([128, G], f16)
            nen = singles.tile([128, NSLABS * W], f32)
            # stationaries + first db/nen column blocks first so the first
            # slab's matmuls and window start ~8us in; the rest streams in
            # under compute.
            nc.sync.dma_start(hq0[:], hq0_d[:])
            nc.sync.dma_start(hqm[:], hqm_d[:])
            nc.sync.dma_start(hq1[:], hq1_d[:])
            nb = PTILE
            for b in range(G // nb):
                s = slice(nb * b, nb * (b + 1))
                nc.sync.dma_start(h0[:, s], h0_d[:, s])
                nc.sync.dma_start(l0[:, s], l0_d[:, s])
                nc.sync.dma_start(h1[:, s], h1_d[:, s])
                if b * W < NSLABS * W:
                    sn = slice(W * b * 4, min(W * (b + 1) * 4, NSLABS * W))
                    nc.sync.dma_start(nen[:, sn], nen_d[:, sn])

            ones_col = singles.tile([128, 1], f32)
            nc.vector.memset(ones_col[:], 1.0)
            match_acc = singles.tile([128, NSLABS], f32)

            # ---------------- main loop over 16 slabs ----------------
            with (
                tc.tile_pool(name="small", bufs=2) as small,
                tc.tile_pool(name="sgn", bufs=2) as sgnp,
                tc.tile_pool(name="mm", bufs=2, space="PSUM") as mmp,
            ):
                for t in range(NSLABS):
                    sl = slice(128 * t, 128 * (t + 1))
                    mw = small.tile([128, W], f32, tag="mw")
                    mdm = small.tile([128, 1], f32, tag="mdm")
                    sgn4 = small.tile([128, G // PTILE], f32, tag="sgn4")
                    last = t == NSLABS - 1
                    wq = 1 if last else 0  # ptile whose completion enables dm
                    pending = []
                    for q in range(G // PTILE):
                        pm = mmp.tile([128, PTILE], f32, tag="mm")
                        pending.append(pm)
                        for c in range(PTILE // CHUNK):
                            ps = pm[:, CHUNK * c:CHUNK * (c + 1)]
                            cols = slice(
                                PTILE * q + CHUNK * c, PTILE * q + CHUNK * (c + 1)
                            )
                            nc.tensor.matmul(
                                ps, hq0[:, sl], h0[:, cols], start=True, stop=False
                            )
                            nc.tensor.matmul(
                                ps, hqm[:, sl], l0[:, cols], start=False, stop=False
                            )
                            nc.tensor.matmul(
                                ps, hq1[:, sl], h1[:, cols], start=False, stop=True
                            )
                        # ---- window -> -dm (after the ptile(s) holding it) ----
                        wl = 128 * t          # window cols [wl, wl+W)
                        if q == wq:
                            if not last:
                                nc.vector.scalar_tensor_tensor(
                                    out=mw[:], in0=pm[:, wl:wl + W], scalar=-1.0,
                                    in1=nen[:, W * t:W * (t + 1)],
                                    op0=Alu.mult, op1=Alu.add,
                                )
                            else:
                                # window [1920, 2176) spans ptile0 and ptile1
                                nc.vector.scalar_tensor_tensor(
                                    out=mw[:, 0:128], in0=pending[0][:, wl:PTILE],
                                    scalar=-1.0, in1=nen[:, W * t:W * t + 128],
                                    op0=Alu.mult, op1=Alu.add,
                                )
                                nc.vector.scalar_tensor_tensor(
                                    out=mw[:, 128:W], in0=pm[:, 0:128],
                                    scalar=-1.0,
                                    in1=nen[:, W * t + 128:W * (t + 1)],
                                    op0=Alu.mult, op1=Alu.add,
                                )
                            nc.vector.tensor_reduce(
                                mdm[:], mw[:], axis=mybir.AxisListType.X,
                                op=Alu.max,
                            )
                        # ---- fused evacuate+count: sgn = Sign(score - dm) ----
                        if q >= wq:
                            for i, pq in enumerate(pending):
                                sgn = sgnp.tile([128, PTILE], f16, tag="sgn")
                                nc.scalar.activation(
                                    sgn[:], pq[:], Act.Sign, bias=mdm[:],
                                    accum_out=sgn4[:, q - len(pending) + 1 + i:
                                                   q - len(pending) + 2 + i],
                                )
                            pending = []
                    ssum = small.tile([128, 1], f32, tag="ssum")
                    nc.vector.reduce_sum(
                        ssum[:], sgn4[:], axis=mybir.AxisListType.X
                    )
                    nc.vector.tensor_scalar(
                        match_acc[:, t:t + 1], ssum[:], thresh, None,
                        op0=Alu.is_ge,
                    )

            # ---------------- final reduction ----------------
            msum = singles.tile([128, 1], f32)
            nc.vector.reduce_sum(msum[:], match_acc[:], axis=mybir.AxisListType.X)
            with tc.tile_pool(name="fin", bufs=1, space="PSUM") as finp:
                pf = finp.tile([1, 1], f32)
                nc.tensor.matmul(pf[:], ones_col[:], msum[:], start=True, stop=True)
                osb = singles.tile([1, 1], f32)
                nc.scalar.activation(osb[:], pf[:], Act.Copy)
                nc.sync.dma_start(out_d[:], osb[:])

    nc.finalize()
    return nc


def _prepare_inputs(embeddings, labels):
    """Sort each flank by label, build per-core rotated fp16 split inputs."""
    emb = np.ascontiguousarray(np.asarray(embeddings, dtype=np.float32))
    lab = np.asarray(labels)
    in_maps = []
    for f in range(NUM_FLANKS):
        ef = emb[f * G:(f + 1) * G]
        lf = lab[f * G:(f + 1) * G]
        order = np.argsort(lf, kind="stable")
        ef, lf = ef[order], lf[order]
        # window-margin safety: same-label runs must fit in M
        runs = np.diff(
            np.flatnonzero(np.concatenate(([True], lf[1:] != lf[:-1], [True])))
        )
        assert runs.max() <= M, f"label run {runs.max()} exceeds window margin {M}"
        for cc in range(CORES_PER_FLANK):
            r = Q * cc
            # db col j = sorted row (j + r - ROLL) mod G ; query i = col i+ROLL
            db = np.ascontiguousarray(np.roll(ef, ROLL - r, axis=0))
            labdb = np.roll(lf, ROLL - r).astype(np.float32)
            h = db.astype(np.float16)
            low = (db - h.astype(np.float32)).astype(np.float16)
            hT = np.ascontiguousarray(h.T)           # [256, G]
            lT = np.ascontiguousarray(low.T)
            sqb = np.einsum(
                "ij,ij->i", db.astype(np.float64), db.astype(np.float64)
            ).astype(np.float32)
            sh = sqb.astype(np.float16)
            slo = (sqb - sh.astype(np.float32)).astype(np.float16)
            l0 = lT[0:128].copy()
            l0[0, :] = sh                            # +sq rides rows 0,1
            l0[1, :] = slo
            qs = slice(ROLL, ROLL + Q)
            hq0 = np.ascontiguousarray(-2.0 * hT[0:128, qs]).astype(np.float16)
            hq1 = np.ascontiguousarray(-2.0 * hT[128:256, qs]).astype(np.float16)
            hqm = hq0.copy()
            hqm[0:2, :] = np.float16(1.0)
            # negative label-window mask, [128, 16*256]
            nen = np.empty((128, NSLABS * W), dtype=np.float32)
            for t in range(NSLABS):
                winl = labdb[128 * t:128 * t + W]       # window labels
                ql = labdb[128 * t + ROLL:128 * t + ROLL + 128]  # query labels
                ne = BIG * (winl[None, :] != ql[:, None]).astype(np.float32)
                ne[np.arange(128), np.arange(128) + ROLL] += BIG  # self
                nen[:, W * t:W * (t + 1)] = -ne
            in_maps.append({
                "h0": np.ascontiguousarray(hT[0:128]),
                "h1": np.ascontiguousarray(hT[128:256]),
                "l0": l0,
                "hq0": hq0,
                "hqm": hqm,
                "hq1": hq1,
                "nen": nen,
            })
    return in_maps


def kernel(embeddings, labels, flanks, k):
    from concourse.bass_utils import run_bass_kernel_spmd

    k = int(k)
    if ("nc", k) not in _cached:
        _cached[("nc", k)] = _build_program(k)
    nc = _cached[("nc", k)]
    in_maps = _prepare_inputs(embeddings, labels)
    res = run_bass_kernel_spmd(nc, in_maps, list(range(NCORES)))
    total = sum(float(r["out"][0, 0]) for r in res.results)
    return np.float32(total / N)


if __name__ == "__main__":
    sys.path.insert(0, os.path.dirname(os.path.abspath(__file__)))
    from reference import setup_inputs, reference

    inputs = setup_inputs()
    expected = float(reference(**inputs))
    got = float(kernel(**{kk: np.asarray(v) for kk, v in inputs.items()}))
    rel = abs(got - expected) / abs(expected)
    print(f"expected={expected} got={got} rel={rel:.3e}")


# revision 5
# speedup vs baseline: 1.8994x; 1.0534x over previous
"""CMC@k accuracy kernel for Trainium2 (8 NeuronCores, SPMD).

Algorithm (per flank of G=8192 rows, D=256, k=5):
  reference = mean over rows of [any of the k nearest neighbours (excl. self)
  shares the row's label].

Reformulation that avoids argsort: for row i let
    score[i,j] = sq[j] - 2*dot[i,j]        (= dist[i,j] - sq[i], same ordering)
    dm[i]      = min over same-label j!=i of score[i,j]
    ssum[i]    = sum_j sign(score[i,j] - dm[i])   (= #greater - #less)
  match[i] <=> cnt <= k where cnt = #less = (G - ties - ssum)/2, i.e.
  match[i] <=> ssum >= G - 1 - 2k  (ties == 1: the defining neighbour; the
  threshold is parity-robust to the HW's sign(0) convention).

Host-side marshalling: each flank is sorted by label (the metric is
permutation invariant), so same-label points are contiguous and the masked
min only needs a narrow 256-column window around the diagonal.  Each of the
4 cores per flank gets the sorted flank rotated so its own 2048 query rows
sit at db columns 64..2112 (the +64 roll makes the label window for query
slab t exactly db cols [128t, 128t+256) -- it never wraps).

Precision: fp32 matmuls are slow on PE (4 passes); instead e is split
e = h + l into fp16 high/low halves and score is built from THREE
single-pass fp16 matmuls per 512-column chunk:
    (-2 h_q0).h_db0  +  hqm.l0  +  (-2 h_q1).h_db1
where l0 rows 0,1 are replaced by the fp16 split of +||e_j||^2 and hqm is
-2 h_q0 with rows 0,1 set to 1.0 (so PSUM = sq[j] - 2*dot directly; the -2
prescale of the stationary operands is exact in fp16).  The dropped
low-order terms (query-low x db, and db-low of half 1) shift scores by
~8e-3, verified on the reference inputs to flip zero match decisions
(margins between k-th/k+1-th neighbour distances are O(1)).

Device schedule (ptile-major so the initial DMA hides behind compute:
db column-block p=0 for all 16 slabs only needs 1/4 of the database):
  warmup: ~20 dummy matmuls on a zero tile while DMA streams (HAM ramp)
  phase p=0, slab t:  PE 12 fp16 matmuls (window chunks first)
                      DVE mw = -psum_win + nen; mdm[t] = max(mw) = -dm
                      ACT sgn = Sign(psum + mdm[t]), accum -> sign-sum
                      (evacuation and counting fused in one PSUM pass)
  phases p=1..3:      PE matmuls + ACT Sign-evac only
  tail: ssum = sum of 4 phase sign-sums; match = (ssum >= G-1-2k);
        count matches -> [1,1] output; host sums and divides by N.
"""
import os
import sys
import numpy as np

sys.path.insert(0, "/opt/trn_rl_repo")

NUM_FLANKS = 2
N, D = 16384, 256
G = N // NUM_FLANKS            # 8192 rows per flank
NCORES = 8
CORES_PER_FLANK = NCORES // NUM_FLANKS
Q = G // CORES_PER_FLANK       # 2048 query rows per core
NSLABS = Q // 128              # 16 slabs per core
M = 64                         # window margin (>= max same-label run)
W = 256                        # window width
ROLL = 64                      # extra db roll so windows never wrap
BIG = 1.0e6
CHUNK = 512                    # matmul free dim (fp32 out, one PSUM bank)
PTILE = 2048                   # psum tile (4 banks)
NPT = G // PTILE               # 4 ptiles
NWARM = 20                     # HAM warm-up matmuls

_cached = {}


def _build_program(k: int):
    import concourse.bacc as bacc
    import concourse.tile as tile
    from concourse import mybir

    f32 = mybir.dt.float32
    f16 = mybir.dt.float16
    Alu = mybir.AluOpType
    Act = mybir.ActivationFunctionType

    nc = bacc.Bacc()
    h0_d = nc.dram_tensor("h0", [128, G], f16, kind="ExternalInput")
    h1_d = nc.dram_tensor("h1", [128, G], f16, kind="ExternalInput")
    l0_d = nc.dram_tensor("l0", [128, G], f16, kind="ExternalInput")
    hq0_d = nc.dram_tensor("hq0", [128, Q], f16, kind="ExternalInput")
    hqm_d = nc.dram_tensor("hqm", [128, Q], f16, kind="ExternalInput")
    hq1_d = nc.dram_tensor("hq1", [128, Q], f16, kind="ExternalInput")
    nen_d = nc.dram_tensor("nen", [128, NSLABS * W], f32, kind="ExternalInput")
    out_d = nc.dram_tensor("out", [1, 1], f32, kind="ExternalOutput")

    thresh = float(G - 1 - 2 * k)

    with tile.TileContext(nc) as tc:
        with tc.tile_pool(name="singles", bufs=1) as singles:
            hq0 = singles.tile([128, Q], f16)
            hqm = singles.tile([128, Q], f16)
            hq1 = singles.tile([128, Q], f16)
            h0 = singles.tile([128, G], f16)
            h1 = singles.tile([128, G], f16)
            l0 = singles.tile([128, G], f16)
            nen = singles.tile([128, NSLABS * W], f32)
            # ---- DMA priority order: the first (t=0, p=0) window chunk
            # needs hq*[:,0:128], db[:,0:512], nen[:,0:256]; then the rest
            # of phase 0; later column blocks stream under compute.
            for d_t, s_t, sl_ in (
                (hq0_d, hq0, slice(0, 128)),
                (hqm_d, hqm, slice(0, 128)),
                (hq1_d, hq1, slice(0, 128)),
                (h0_d, h0, slice(0, CHUNK)),
                (l0_d, l0, slice(0, CHUNK)),
                (h1_d, h1, slice(0, CHUNK)),
                (nen_d, nen, slice(0, W)),
                (hq0_d, hq0, slice(128, Q)),
                (hqm_d, hqm, slice(128, Q)),
                (hq1_d, hq1, slice(128, Q)),
                (h0_d, h0, slice(CHUNK, PTILE)),
                (l0_d, l0, slice(CHUNK, PTILE)),
                (h1_d, h1, slice(CHUNK, PTILE)),
                (nen_d, nen, slice(W, NSLABS * W)),
            ):
                nc.sync.dma_start(s_t[:, sl_], d_t[:, sl_])
            for b in range(1, NPT):
                s = slice(PTILE * b, PTILE * (b + 1))
                nc.sync.dma_start(h0[:, s], h0_d[:, s])
                nc.sync.dma_start(l0[:, s], l0_d[:, s])
                nc.sync.dma_start(h1[:, s], h1_d[:, s])

            ones_col = singles.tile([128, 1], f32)
            nc.vector.memset(ones_col[:], 1.0)
            wtile = singles.tile([128, CHUNK + 128], f16)
            nc.vector.memset(wtile[:], 0.0)
            wsink = singles.tile([1, 1], f32)
            mdm_all = singles.tile([128, NSLABS], f32)
            sgn_all = singles.tile([128, NPT * NSLABS], f32)

            with (
                tc.tile_pool(name="small", bufs=2) as small,
                tc.tile_pool(name="sgn", bufs=2) as sgnp,
                tc.tile_pool(name="mm", bufs=2, space="PSUM") as mmp,
            ):
                # ---- HAM warm-up: keep PE busy while DMA streams ----
                wpm = mmp.tile([128, PTILE], f32, tag="mm")
                for i in range(NWARM):
                    nc.tensor.matmul(
                        wpm[:, 0:CHUNK], wtile[:, 0:128],
                        wtile[:, 128:128 + CHUNK], start=True, stop=True,
                    )
                nc.scalar.activation(wsink[:], wpm[0:1, 0:1], Act.Copy)

                def mm_ptile(pm, t, p, order):
                    sl = slice(128 * t, 128 * (t + 1))
                    for c in order:
                        ps = pm[:, CHUNK * c:CHUNK * (c + 1)]
                        cols = slice(
                            PTILE * p + CHUNK * c, PTILE * p + CHUNK * (c + 1)
                        )
                        nc.tensor.matmul(
                            ps, hq0[:, sl], h0[:, cols], start=True, stop=False
                        )
                        nc.tensor.matmul(
                            ps, hqm[:, sl], l0[:, cols], start=False, stop=False
                        )
                        nc.tensor.matmul(
                            ps, hq1[:, sl], h1[:, cols], start=False, stop=True
                        )

                def sign_evac(pm, t, p):
                    sgn = sgnp.tile([128, PTILE], f16, tag="sgn")
                    nc.scalar.activation(
                        sgn[:], pm[:], Act.Sign, bias=mdm_all[:, t:t + 1],
                        accum_out=sgn_all[:, NSLABS * p + t:NSLABS * p + t + 1],
                    )

                # ---- phase 0: window+mdm per slab, then evac ----
                for t in range(NSLABS):
                    wl = 128 * t
                    c_lo, c_hi = wl // CHUNK, min((wl + W - 1) // CHUNK, 3)
                    order = [c_lo] + ([c_hi] if c_hi != c_lo else [])
                    order += [c for c in range(NPT) if c not in order]
                    pm = mmp.tile([128, PTILE], f32, tag="mm")
                    mm_ptile(pm, t, 0, order)
                    mw = small.tile([128, W], f32, tag="mw")
                    if t < NSLABS - 1:
                        nc.vector.scalar_tensor_tensor(
                            out=mw[:], in0=pm[:, wl:wl + W], scalar=-1.0,
                            in1=nen[:, W * t:W * (t + 1)],
                            op0=Alu.mult, op1=Alu.add,
                        )
                        nc.vector.tensor_reduce(
                            mdm_all[:, t:t + 1], mw[:],
                            axis=mybir.AxisListType.X, op=Alu.max,
                        )
                        sign_evac(pm, t, 0)
                    else:
                        # window [1920, 2176) spans ptile 0 and ptile 1:
                        # compute ptile 1 now too, then evac both.
                        pm1 = mmp.tile([128, PTILE], f32, tag="mm")
                        mm_ptile(pm1, t, 1, list(range(NPT)))
                        nc.vector.scalar_tensor_tensor(
                            out=mw[:, 0:128], in0=pm[:, wl:PTILE], scalar=-1.0,
                            in1=nen[:, W * t:W * t + 128],
                            op0=Alu.mult, op1=Alu.add,
                        )
                        nc.vector.scalar_tensor_tensor(
                            out=mw[:, 128:W], in0=pm1[:, 0:128], scalar=-1.0,
                            in1=nen[:, W * t + 128:W * (t + 1)],
                            op0=Alu.mult, op1=Alu.add,
                        )
                        nc.vector.tensor_reduce(
                            mdm_all[:, t:t + 1], mw[:],
                            axis=mybir.AxisListType.X, op=Alu.max,
                        )
                        sign_evac(pm, t, 0)
                        sign_evac(pm1, t, 1)

                # ---- phases 1..3: matmul + fused Sign evac only ----
                for p in range(1, NPT):
                    for t in range(NSLABS):
                        if p == 1 and t == NSLABS - 1:
                            continue  # done in phase 0
                        pm = mmp.tile([128, PTILE], f32, tag="mm")
                        mm_ptile(pm, t, p, list(range(NPT)))
                        sign_evac(pm, t, p)

                # ---- combine sign-sums; match = (ssum >= G-1-2k) ----
                s01 = small.tile([128, NSLABS], f32, tag="s01")
                s23 = small.tile([128, NSLABS], f32, tag="s23")
                match16 = small.tile([128, NSLABS], f32, tag="match")
                nc.vector.tensor_tensor(
                    out=s01[:], in0=sgn_all[:, 0:NSLABS],
                    in1=sgn_all[:, NSLABS:2 * NSLABS], op=Alu.add,
                )
                nc.vector.tensor_tensor(
                    out=s23[:], in0=sgn_all[:, 2 * NSLABS:3 * NSLABS],
                    in1=sgn_all[:, 3 * NSLABS:4 * NSLABS], op=Alu.add,
                )
                nc.vector.tensor_tensor(
                    out=s01[:], in0=s01[:], in1=s23[:], op=Alu.add,
                )
                nc.vector.tensor_scalar(
                    match16[:], s01[:], thresh, None, op0=Alu.is_ge,
                )
                msum = small.tile([128, 1], f32, tag="msum")
                nc.vector.reduce_sum(
                    msum[:], match16[:], axis=mybir.AxisListType.X
                )

            with tc.tile_pool(name="fin", bufs=1, space="PSUM") as finp:
                pf = finp.tile([1, 1], f32)
                nc.tensor.matmul(pf[:], ones_col[:], msum[:], start=True, stop=True)
                osb = singles.tile([1, 1], f32)
                nc.scalar.activation(osb[:], pf[:], Act.Copy)
                nc.sync.dma_start(out_d[:], osb[:])

    nc.finalize()
    return nc


def _prepare_inputs(embeddings, labels):
    """Sort each flank by label, build per-core rotated fp16 split inputs."""
    emb = np.ascontiguousarray(np.asarray(embeddings, dtype=np.float32))
    lab = np.asarray(labels)
    in_maps = []
    for f in range(NUM_FLANKS):
        ef = emb[f * G:(f + 1) * G]
        lf = lab[f * G:(f + 1) * G]
        order = np.argsort(lf, kind="stable")
        ef, lf = ef[order], lf[order]
        # window-margin safety: same-label runs must fit in M
        runs = np.diff(
            np.flatnonzero(np.concatenate(([True], lf[1:] != lf[:-1], [True])))
        )
        assert runs.max() <= M, f"label run {runs.max()} exceeds window margin {M}"
        for cc in range(CORES_PER_FLANK):
            r = Q * cc
            # db col j = sorted row (j + r - ROLL) mod G ; query i = col i+ROLL
            db = np.ascontiguousarray(np.roll(ef, ROLL - r, axis=0))
            labdb = np.roll(lf, ROLL - r).astype(np.float32)
            h = db.astype(np.float16)
            low = (db - h.astype(np.float32)).astype(np.float16)
            hT = np.ascontiguousarray(h.T)           # [256, G]
            lT = np.ascontiguousarray(low.T)
            sqb = np.einsum(
                "ij,ij->i", db.astype(np.float64), db.astype(np.float64)
            ).astype(np.float32)
            sh = sqb.astype(np.float16)
            slo = (sqb - sh.astype(np.float32)).astype(np.float16)
            l0 = lT[0:128].copy()
            l0[0, :] = sh                            # +sq rides rows 0,1
            l0[1, :] = slo
            qs = slice(ROLL, ROLL + Q)
            hq0 = np.ascontiguousarray(-2.0 * hT[0:128, qs]).astype(np.float16)
            hq1 = np.ascontiguousarray(-2.0 * hT[128:256, qs]).astype(np.float16)
            hqm = hq0.copy()
            hqm[0:2, :] = np.float16(1.0)
            # negative label-window mask, [128, 16*256]
            nen = np.empty((128, NSLABS * W), dtype=np.float32)
            for t in range(NSLABS):
                winl = labdb[128 * t:128 * t + W]       # window labels
                ql = labdb[128 * t + ROLL:128 * t + ROLL + 128]  # query labels
                ne = BIG * (winl[None, :] != ql[:, None]).astype(np.float32)
                ne[np.arange(128), np.arange(128) + ROLL] += BIG  # self
                nen[:, W * t:W * (t + 1)] = -ne
            in_maps.append({
                "h0": np.ascontiguousarray(hT[0:128]),
                "h1": np.ascontiguousarray(hT[128:256]),
                "l0": l0,
                "hq0": hq0,
                "hqm": hqm,
                "hq1": hq1,
                "nen": nen,
            })
    return in_maps


def kernel(embeddings, labels, flanks, k):
    from concourse.bass_utils import run_bass_kernel_spmd

    k = int(k)
    if ("nc", k) not in _cached:
        _cached[("nc", k)] = _build_program(k)
    nc = _cached[("nc", k)]
    in_maps = _prepare_inputs(embeddings, labels)
    res = run_bass_kernel_spmd(nc, in_maps, list(range(NCORES)))
    total = sum(float(r["out"][0, 0]) for r in res.results)
    return np.float32(total / N)


if __name__ == "__main__":
    sys.path.insert(0, os.path.dirname(os.path.abspath(__file__)))
    from reference import setup_inputs, reference

    inputs = setup_inputs()
    expected = float(reference(**inputs))
    got = float(kernel(**{kk: np.asarray(v) for kk, v in inputs.items()}))
    rel = abs(got - expected) / abs(got) if got else 1.0
    print(f"expected={expected} got={got} rel={rel:.3e}")


# revision 13
# speedup vs baseline: 1.9797x; 1.0423x over previous
"""CMC@k accuracy kernel for Trainium2 (8 NeuronCores, SPMD).

Algorithm (per flank of G=8192 rows, D=256, k=5):
  reference = mean over rows of [any of the k nearest neighbours (excl. self)
  shares the row's label].

Reformulation that avoids argsort: for row i let
    score[i,j] = sq[j] - 2*dot[i,j]        (= dist[i,j] - sq[i], same ordering)
    dm[i]      = min over same-label j!=i of score[i,j]
    ssum[i]    = sum_j sign(score[i,j] - dm[i])   (= #greater - #less)
  match[i] <=> cnt <= k where cnt = #less = (G - ties - ssum)/2, i.e.
  match[i] <=> ssum >= G - 1 - 2k  (ties == 1: the defining neighbour; the
  threshold is parity-robust to the HW's sign(0) convention).

Host-side marshalling: each flank is sorted by label (the metric is
permutation invariant), so same-label points are contiguous and the masked
min only needs a narrow 256-column window around the diagonal.  Each of the
4 cores per flank gets the sorted flank rotated so its own 2048 query rows
sit at db columns 64..2112 (the +64 roll makes the label window for query
slab t exactly db cols [128t, 128t+256) -- it never wraps).

Precision: fp32 matmuls are slow on PE (4 passes); instead e is split
e = h + l into fp16 high/low halves and score is built from THREE
single-pass fp16 matmuls per 512-column chunk:
    (-2 h_q0).h_db0  +  hqm.l0  +  (-2 h_q1).h_db1
where l0 rows 0,1 are replaced by the fp16 split of +||e_j||^2 and hqm is
-2 h_q0 with rows 0,1 set to 1.0 (so PSUM = sq[j] - 2*dot directly; the -2
prescale of the stationary operands is exact in fp16).  The dropped
low-order terms (query-low x db, and db-low of half 1) shift scores by
~8e-3, verified on the reference inputs to flip zero match decisions
(margins between k-th/k+1-th neighbour distances are O(1)).

Device schedule (ptile-major so the initial DMA hides behind compute:
db column-block p=0 for all 16 slabs only needs 1/4 of the database):
  warmup: ~20 dummy matmuls on a zero tile while DMA streams (HAM ramp)
  phase p=0, slab t:  PE 12 fp16 matmuls (window chunks first)
                      DVE mw = -psum_win + nen; mdm[t] = max(mw) = -dm
                      ACT sgn = Sign(psum + mdm[t]), accum -> sign-sum
                      (evacuation and counting fused in one PSUM pass)
  phases p=1..3:      PE matmuls + ACT Sign-evac only
  tail: ssum = sum of 4 phase sign-sums; match = (ssum >= G-1-2k);
        count matches -> [1,1] output; host sums and divides by N.
"""
import os
import sys
import numpy as np

sys.path.insert(0, "/opt/trn_rl_repo")

NUM_FLANKS = 2
N, D = 16384, 256
G = N // NUM_FLANKS            # 8192 rows per flank
NCORES = 8
CORES_PER_FLANK = NCORES // NUM_FLANKS
Q = G // CORES_PER_FLANK       # 2048 query rows per core
NSLABS = Q // 128              # 16 slabs per core
M = 64                         # window margin (>= max same-label run)
W = 256                        # window width
ROLL = 64                      # extra db roll so windows never wrap
BIG = 1.0e6
CHUNK = 512                    # matmul free dim (fp32 out, one PSUM bank)
PTILE = 2048                   # logical column block per phase
HALF = 1024                    # psum tile (2 banks; 4 tiles in flight)
NPT = G // PTILE               # 4 phases
NSUB = G // HALF               # 8 evacuation subtiles per slab
NDVE = 2                       # slabs evacuated by DVE (count convention)

_cached = {}


def _build_program(k: int):
    import concourse.bacc as bacc
    import concourse.tile as tile
    from concourse import mybir

    f32 = mybir.dt.float32
    f16 = mybir.dt.float16
    Alu = mybir.AluOpType
    Act = mybir.ActivationFunctionType

    nc = bacc.Bacc()
    h0_d = nc.dram_tensor("h0", [128, G], f16, kind="ExternalInput")
    h1_d = nc.dram_tensor("h1", [128, G], f16, kind="ExternalInput")
    l0_d = nc.dram_tensor("l0", [128, G], f16, kind="ExternalInput")
    hq0_d = nc.dram_tensor("hq0", [128, Q], f16, kind="ExternalInput")
    hqm_d = nc.dram_tensor("hqm", [128, Q], f16, kind="ExternalInput")
    hq1_d = nc.dram_tensor("hq1", [128, Q], f16, kind="ExternalInput")
    nen_d = nc.dram_tensor("nen", [128, NSLABS * W], f32, kind="ExternalInput")
    out_d = nc.dram_tensor("out", [1, 1], f32, kind="ExternalOutput")

    thresh = float(G - 1 - 2 * k)

    with tile.TileContext(nc) as tc:
        with tc.tile_pool(name="singles", bufs=1) as singles:
            hq0 = singles.tile([128, Q], f16)
            hqm = singles.tile([128, Q], f16)
            hq1 = singles.tile([128, Q], f16)
            h0 = singles.tile([128, G], f16)
            h1 = singles.tile([128, G], f16)
            l0 = singles.tile([128, G], f16)
            nen = singles.tile([128, NSLABS * W], f32)
            # ---- DMA priority order: the first (t=0, p=0) window chunk
            # needs hq*[:,0:128], db[:,0:512], nen[:,0:256]; then the rest
            # of phase 0; later column blocks stream under compute.
            for d_t, s_t, sl_ in (
                (hq0_d, hq0, slice(0, 128)),
                (hqm_d, hqm, slice(0, 128)),
                (hq1_d, hq1, slice(0, 128)),
                (h0_d, h0, slice(0, CHUNK)),
                (l0_d, l0, slice(0, CHUNK)),
                (h1_d, h1, slice(0, CHUNK)),
                (nen_d, nen, slice(0, W)),
                (hq0_d, hq0, slice(128, Q)),
                (hqm_d, hqm, slice(128, Q)),
                (hq1_d, hq1, slice(128, Q)),
                (h0_d, h0, slice(CHUNK, PTILE)),
                (l0_d, l0, slice(CHUNK, PTILE)),
                (h1_d, h1, slice(CHUNK, PTILE)),
                (nen_d, nen, slice(W, NSLABS * W)),
            ):
                nc.sync.dma_start(s_t[:, sl_], d_t[:, sl_])
            for b in range(1, NPT):
                s = slice(PTILE * b, PTILE * (b + 1))
                nc.sync.dma_start(h0[:, s], h0_d[:, s])
                nc.sync.dma_start(l0[:, s], l0_d[:, s])
                nc.sync.dma_start(h1[:, s], h1_d[:, s])

            ones_col = singles.tile([128, 1], f32)
            nc.vector.memset(ones_col[:], 1.0)
            mdm_all = singles.tile([128, NSLABS], f32)
            dmq = singles.tile([128, NDVE], f32)  # +dm for DVE-evac slabs
            sgn_all = singles.tile([128, NSUB * NSLABS], f32)

            with (
                tc.tile_pool(name="small", bufs=2) as small,
                tc.tile_pool(name="sgn", bufs=2) as sgnp,
                tc.tile_pool(name="mm", bufs=4, space="PSUM") as mmp,
            ):
                def mm_half(t, sub):
                    """3-pass matmuls for query slab t, db cols
                    [HALF*sub, HALF*(sub+1)); returns the psum tile."""
                    pm = mmp.tile([128, HALF], f32, tag="mm")
                    sl = slice(128 * t, 128 * (t + 1))
                    for c in range(HALF // CHUNK):
                        ps = pm[:, CHUNK * c:CHUNK * (c + 1)]
                        cols = slice(
                            HALF * sub + CHUNK * c, HALF * sub + CHUNK * (c + 1)
                        )
                        nc.tensor.matmul(
                            ps, hq0[:, sl], h0[:, cols], start=True, stop=False
                        )
                        nc.tensor.matmul(
                            ps, hqm[:, sl], l0[:, cols], start=False, stop=False
                        )
                        nc.tensor.matmul(
                            ps, hq1[:, sl], h1[:, cols], start=False, stop=True
                        )
                    return pm

                def sign_evac(pm, t, sub):
                    sgn = sgnp.tile([128, HALF], f16, tag="sgn")
                    col = NSLABS * sub + t
                    if t < NDVE:
                        # DVE evac, count convention: accum = #{score < dm}
                        nc.vector.tensor_scalar(
                            sgn[:], pm[:], dmq[:, t:t + 1], None,
                            op0=Alu.is_lt, op1=Alu.add,
                            accum_out=sgn_all[:, col:col + 1],
                        )
                    else:
                        # ACT evac, sign convention: accum = #gt - #lt
                        nc.scalar.activation(
                            sgn[:], pm[:], Act.Sign, bias=mdm_all[:, t:t + 1],
                            accum_out=sgn_all[:, col:col + 1],
                        )

                def window_stt(mw, lo, n, pm, off, t):
                    nc.vector.scalar_tensor_tensor(
                        out=mw[:, lo:lo + n], in0=pm[:, off:off + n],
                        scalar=-1.0, in1=nen[:, W * t + lo:W * t + lo + n],
                        op0=Alu.mult, op1=Alu.add,
                    )

                # ---- phase 0 (db cols [0, 2048) + slab 15's extra half):
                # window -> mdm per slab, then fused Sign evacuation.
                for t in range(NSLABS):
                    wl = 128 * t
                    pmA = mm_half(t, 0)
                    pmB = mm_half(t, 1)
                    halves = [pmA, pmB]
                    mw = small.tile([128, W], f32, tag="mw")
                    if t == NSLABS - 1:
                        # window [1920, 2176) needs cols [2048, 2176) too
                        pmC = mm_half(t, 2)
                        halves.append(pmC)
                        window_stt(mw, 0, 128, pmB, wl - HALF, t)
                        window_stt(mw, 128, 128, pmC, 0, t)
                    elif wl + W <= HALF:
                        window_stt(mw, 0, W, pmA, wl, t)
                    elif wl >= HALF:
                        window_stt(mw, 0, W, pmB, wl - HALF, t)
                    else:  # t == 7: [896, 1152) spans both halves
                        window_stt(mw, 0, HALF - wl, pmA, wl, t)
                        window_stt(mw, HALF - wl, wl + W - HALF, pmB, 0, t)
                    nc.vector.tensor_reduce(
                        mdm_all[:, t:t + 1], mw[:],
                        axis=mybir.AxisListType.X, op=Alu.max,
                    )
                    if t < NDVE:
                        nc.vector.tensor_scalar(
                            dmq[:, t:t + 1], mdm_all[:, t:t + 1], -1.0, None,
                            op0=Alu.mult,
                        )
                    for i, pm in enumerate(halves):
                        sign_evac(pm, t, i)

                # ---- remaining column blocks: matmul + Sign evac only ----
                for sub in range(2, NSUB):
                    for t in range(NSLABS):
                        if sub == 2 and t == NSLABS - 1:
                            continue  # done in phase 0
                        pm = mm_half(t, sub)
                        sign_evac(pm, t, sub)

                # ---- combine sign-sums; match = (ssum >= G-1-2k) ----
                acc = small.tile([128, NSLABS], f32, tag="acc")
                nc.vector.tensor_tensor(
                    out=acc[:], in0=sgn_all[:, 0:NSLABS],
                    in1=sgn_all[:, NSLABS:2 * NSLABS], op=Alu.add,
                )
                for sub in range(2, NSUB):
                    nc.vector.tensor_tensor(
                        out=acc[:], in0=acc[:],
                        in1=sgn_all[:, NSLABS * sub:NSLABS * (sub + 1)],
                        op=Alu.add,
                    )
                match16 = small.tile([128, NSLABS], f32, tag="match")
                # DVE-evacuated slabs hold counts: match <=> cnt <= k;
                # ACT slabs hold sign-sums: match <=> ssum >= G-1-2k.
                nc.vector.tensor_scalar(
                    match16[:, 0:NDVE], acc[:, 0:NDVE], float(k), None,
                    op0=Alu.is_le,
                )
                nc.vector.tensor_scalar(
                    match16[:, NDVE:], acc[:, NDVE:], thresh, None,
                    op0=Alu.is_ge,
                )
                msum = small.tile([128, 1], f32, tag="msum")
                nc.vector.reduce_sum(
                    msum[:], match16[:], axis=mybir.AxisListType.X
                )

            with tc.tile_pool(name="fin", bufs=1, space="PSUM") as finp:
                pf = finp.tile([1, 1], f32)
                nc.tensor.matmul(pf[:], ones_col[:], msum[:], start=True, stop=True)
                osb = singles.tile([1, 1], f32)
                nc.scalar.activation(osb[:], pf[:], Act.Copy)
                nc.sync.dma_start(out_d[:], osb[:])

    nc.finalize()
    return nc


def _prepare_inputs(embeddings, labels):
    """Sort each flank by label, build per-core rotated fp16 split inputs."""
    emb = np.ascontiguousarray(np.asarray(embeddings, dtype=np.float32))
    lab = np.asarray(labels)
    in_maps = []
    for f in range(NUM_FLANKS):
        ef = emb[f * G:(f + 1) * G]
        lf = lab[f * G:(f + 1) * G]
        order = np.argsort(lf, kind="stable")
        ef, lf = ef[order], lf[order]
        # window-margin safety: same-label runs must fit in M
        runs = np.diff(
            np.flatnonzero(np.concatenate(([True], lf[1:] != lf[:-1], [True])))
        )
        assert runs.max() <= M, f"label run {runs.max()} exceeds window margin {M}"
        for cc in range(CORES_PER_FLANK):
            r = Q * cc
            # db col j = sorted row (j + r - ROLL) mod G ; query i = col i+ROLL
            db = np.ascontiguousarray(np.roll(ef, ROLL - r, axis=0))
            labdb = np.roll(lf, ROLL - r).astype(np.float32)
            h = db.astype(np.float16)
            low = (db - h.astype(np.float32)).astype(np.float16)
            hT = np.ascontiguousarray(h.T)           # [256, G]
            lT = np.ascontiguousarray(low.T)
            sqb = np.einsum(
                "ij,ij->i", db.astype(np.float64), db.astype(np.float64)
            ).astype(np.float32)
            sh = sqb.astype(np.float16)
            slo = (sqb - sh.astype(np.float32)).astype(np.float16)
            l0 = lT[0:128].copy()
            l0[0, :] = sh                            # +sq rides rows 0,1
            l0[1, :] = slo
            qs = slice(ROLL, ROLL + Q)
            hq0 = np.ascontiguousarray(-2.0 * hT[0:128, qs]).astype(np.float16)
            hq1 = np.ascontiguousarray(-2.0 * hT[128:256, qs]).astype(np.float16)
            hqm = hq0.copy()
            hqm[0:2, :] = np.float16(1.0)
            # negative label-window mask, [128, 16*256]
            nen = np.empty((128, NSLABS * W), dtype=np.float32)
            for t in range(NSLABS):
                winl = labdb[128 * t:128 * t + W]       # window labels
                ql = labdb[128 * t + ROLL:128 * t + ROLL + 128]  # query labels
                ne = BIG * (winl[None, :] != ql[:, None]).astype(np.float32)
                ne[np.arange(128), np.arange(128) + ROLL] += BIG  # self
                nen[:, W * t:W * (t + 1)] = -ne
            in_maps.append({
                "h0": np.ascontiguousarray(hT[0:128]),
                "h1": np.ascontiguousarray(hT[128:256]),
                "l0": l0,
                "hq0": hq0,
                "hqm": hqm,
                "hq1": hq1,
                "nen": nen,
            })
    return in_maps


def kernel(embeddings, labels, flanks, k):
    from concourse.bass_utils import run_bass_kernel_spmd

    k = int(k)
    if ("nc", k) not in _cached:
        _cached[("nc", k)] = _build_program(k)
    nc = _cached[("nc", k)]
    in_maps = _prepare_inputs(embeddings, labels)
    res = run_bass_kernel_spmd(nc, in_maps, list(range(NCORES)))
    total = sum(float(r["out"][0, 0]) for r in res.results)
    return np.float32(total / N)


if __name__ == "__main__":
    sys.path.insert(0, os.path.dirname(os.path.abspath(__file__)))
    from reference import setup_inputs, reference

    inputs = setup_inputs()
    expected = float(reference(**inputs))
    got = float(kernel(**{kk: np.asarray(v) for kk, v in inputs.items()}))
    rel = abs(got - expected) / abs(got) if got else 1.0
    print(f"expected={expected} got={got} rel={rel:.3e}")


# revision 16
# speedup vs baseline: 2.0124x; 1.0165x over previous
"""CMC@k accuracy kernel for Trainium2 (8 NeuronCores, SPMD).

Algorithm (per flank of G=8192 rows, D=256, k=5):
  reference = mean over rows of [any of the k nearest neighbours (excl. self)
  shares the row's label].

Reformulation that avoids argsort: for row i let
    score[i,j] = sq[j] - 2*dot[i,j]        (= dist[i,j] - sq[i], same ordering)
    dm[i]      = min over same-label j!=i of score[i,j]
    ssum[i]    = sum_j sign(score[i,j] - dm[i])   (= #greater - #less)
  match[i] <=> cnt <= k where cnt = #less = (G - ties - ssum)/2, i.e.
  match[i] <=> ssum >= G - 1 - 2k  (ties == 1: the defining neighbour; the
  threshold is parity-robust to the HW's sign(0) convention).

Host-side marshalling: each flank is sorted by label (the metric is
permutation invariant), so same-label points are contiguous and the masked
min only needs a narrow 256-column window around the diagonal.  Each of the
4 cores per flank gets the sorted flank rotated so its own 2048 query rows
sit at db columns 64..2112 (the +64 roll makes the label window for query
slab t exactly db cols [128t, 128t+256) -- it never wraps).

Precision: fp32 matmuls are slow on PE (4 passes); instead e is split
e = h + l into fp16 high/low halves and score is built from THREE
single-pass fp16 matmuls per 512-column chunk:
    (-2 h_q0).h_db0  +  hqm.l0  +  (-2 h_q1).h_db1
where l0 rows 0,1 are replaced by the fp16 split of +||e_j||^2 and hqm is
-2 h_q0 with rows 0,1 set to 1.0 (so PSUM = sq[j] - 2*dot directly; the -2
prescale of the stationary operands is exact in fp16).  The dropped
low-order terms (query-low x db, and db-low of half 1) shift scores by
~8e-3, verified on the reference inputs to flip zero match decisions
(margins between k-th/k+1-th neighbour distances are O(1)).

Device schedule (ptile-major so the initial DMA hides behind compute:
db column-block p=0 for all 16 slabs only needs 1/4 of the database):
  warmup: ~20 dummy matmuls on a zero tile while DMA streams (HAM ramp)
  phase p=0, slab t:  PE 12 fp16 matmuls (window chunks first)
                      DVE mw = -psum_win + nen; mdm[t] = max(mw) = -dm
                      ACT sgn = Sign(psum + mdm[t]), accum -> sign-sum
                      (evacuation and counting fused in one PSUM pass)
  phases p=1..3:      PE matmuls + ACT Sign-evac only
  tail: ssum = sum of 4 phase sign-sums; match = (ssum >= G-1-2k);
        count matches -> [1,1] output; host sums and divides by N.
"""
import os
import sys
import numpy as np

sys.path.insert(0, "/opt/trn_rl_repo")

NUM_FLANKS = 2
N, D = 16384, 256
G = N // NUM_FLANKS            # 8192 rows per flank
NCORES = 8
CORES_PER_FLANK = NCORES // NUM_FLANKS
Q = G // CORES_PER_FLANK       # 2048 query rows per core
NSLABS = Q // 128              # 16 slabs per core
M = 64                         # window margin (>= max same-label run)
W = 256                        # window width
ROLL = 64                      # extra db roll so windows never wrap
BIG = 1.0e6
CHUNK = 512                    # matmul free dim (fp32 out, one PSUM bank)
PTILE = 2048                   # logical column block per phase
HALF = 1024                    # psum tile (2 banks; 4 tiles in flight)
NPT = G // PTILE               # 4 phases
NSUB = G // HALF               # 8 evacuation subtiles per slab
NDVE = 2                       # slabs evacuated by DVE (count convention)

_cached = {}


def _build_program(k: int):
    import concourse.bacc as bacc
    import concourse.tile as tile
    from concourse import mybir

    f32 = mybir.dt.float32
    f16 = mybir.dt.float16
    Alu = mybir.AluOpType
    Act = mybir.ActivationFunctionType

    nc = bacc.Bacc()
    h0_d = nc.dram_tensor("h0", [128, G], f16, kind="ExternalInput")
    h1_d = nc.dram_tensor("h1", [128, G], f16, kind="ExternalInput")
    l0_d = nc.dram_tensor("l0", [128, G], f16, kind="ExternalInput")
    hq0_d = nc.dram_tensor("hq0", [128, Q], f16, kind="ExternalInput")
    hqm_d = nc.dram_tensor("hqm", [128, Q], f16, kind="ExternalInput")
    hq1_d = nc.dram_tensor("hq1", [128, Q], f16, kind="ExternalInput")
    nen_d = nc.dram_tensor("nen", [128, NSLABS * W], f32, kind="ExternalInput")
    out_d = nc.dram_tensor("out", [1, 1], f32, kind="ExternalOutput")

    thresh = float(G - 1 - 2 * k)

    with tile.TileContext(nc) as tc:
        with tc.tile_pool(name="singles", bufs=1) as singles:
            hq0 = singles.tile([128, Q], f16)
            hqm = singles.tile([128, Q], f16)
            hq1 = singles.tile([128, Q], f16)
            h0 = singles.tile([128, G], f16)
            h1 = singles.tile([128, G], f16)
            l0 = singles.tile([128, G], f16)
            nen = singles.tile([128, NSLABS * W], f32)
            # ---- DMA priority order, matched to phase-0 consumption:
            # slab 0 needs hq*[:,0:128], db[:,0:2048], nen[:,0:256]; slab t
            # adds hq*[:,128t:...] and nen[:,256t:...]; later column blocks
            # stream under compute.
            dma_list = [
                (hq0_d, hq0, slice(0, 128)),
                (hqm_d, hqm, slice(0, 128)),
                (hq1_d, hq1, slice(0, 128)),
                (h0_d, h0, slice(0, HALF)),
                (l0_d, l0, slice(0, HALF)),
                (h1_d, h1, slice(0, HALF)),
                (nen_d, nen, slice(0, 2 * W)),
                (h0_d, h0, slice(HALF, PTILE)),
                (l0_d, l0, slice(HALF, PTILE)),
                (h1_d, h1, slice(HALF, PTILE)),
            ]
            for t in range(1, NSLABS):
                sq = slice(128 * t, 128 * (t + 1))
                dma_list += [(hq0_d, hq0, sq), (hqm_d, hqm, sq),
                             (hq1_d, hq1, sq)]
                if t >= 2:
                    dma_list.append((nen_d, nen, slice(W * t, W * (t + 1))))
            for b in range(1, NPT):
                s = slice(PTILE * b, PTILE * (b + 1))
                dma_list += [(h0_d, h0, s), (l0_d, l0, s), (h1_d, h1, s)]
            for d_t, s_t, sl_ in dma_list:
                nc.sync.dma_start(s_t[:, sl_], d_t[:, sl_])

            ones_col = singles.tile([128, 1], f32)
            nc.vector.memset(ones_col[:], 1.0)
            mdm_all = singles.tile([128, NSLABS], f32)
            dmq = singles.tile([128, NDVE], f32)  # +dm for DVE-evac slabs
            acc = singles.tile([128, NSLABS], f32)  # running sign-sum
            sgn_all = singles.tile([128, NSUB * NSLABS], f32)

            with (
                tc.tile_pool(name="small", bufs=2) as small,
                tc.tile_pool(name="sgn", bufs=2) as sgnp,
                tc.tile_pool(name="mm", bufs=4, space="PSUM") as mmp,
            ):
                def mm_half(t, sub):
                    """3-pass matmuls for query slab t, db cols
                    [HALF*sub, HALF*(sub+1)); returns the psum tile."""
                    pm = mmp.tile([128, HALF], f32, tag="mm")
                    sl = slice(128 * t, 128 * (t + 1))
                    for c in range(HALF // CHUNK):
                        ps = pm[:, CHUNK * c:CHUNK * (c + 1)]
                        cols = slice(
                            HALF * sub + CHUNK * c, HALF * sub + CHUNK * (c + 1)
                        )
                        nc.tensor.matmul(
                            ps, hq0[:, sl], h0[:, cols], start=True, stop=False
                        )
                        nc.tensor.matmul(
                            ps, hqm[:, sl], l0[:, cols], start=False, stop=False
                        )
                        nc.tensor.matmul(
                            ps, hq1[:, sl], h1[:, cols], start=False, stop=True
                        )
                    return pm

                def sign_evac(pm, t, sub):
                    sgn = sgnp.tile([128, HALF], f16, tag="sgn")
                    col = NSLABS * sub + t
                    if t < NDVE:
                        # DVE evac, count convention: accum = #{score < dm}
                        nc.vector.tensor_scalar(
                            sgn[:], pm[:], dmq[:, t:t + 1], None,
                            op0=Alu.is_lt, op1=Alu.add,
                            accum_out=sgn_all[:, col:col + 1],
                        )
                    else:
                        # ACT evac, sign convention: accum = #gt - #lt
                        nc.scalar.activation(
                            sgn[:], pm[:], Act.Sign, bias=mdm_all[:, t:t + 1],
                            accum_out=sgn_all[:, col:col + 1],
                        )

                def window_stt(mw, lo, n, pm, off, t):
                    nc.vector.scalar_tensor_tensor(
                        out=mw[:, lo:lo + n], in0=pm[:, off:off + n],
                        scalar=-1.0, in1=nen[:, W * t + lo:W * t + lo + n],
                        op0=Alu.mult, op1=Alu.add,
                    )

                # ---- phase 0 (db cols [0, 2048) + slab 15's extra half):
                # window -> mdm per slab, then fused Sign evacuation.
                for t in range(NSLABS):
                    wl = 128 * t
                    pmA = mm_half(t, 0)
                    pmB = mm_half(t, 1)
                    halves = [pmA, pmB]
                    mw = small.tile([128, W], f32, tag="mw")
                    if t == NSLABS - 1:
                        # window [1920, 2176) needs cols [2048, 2176) too
                        pmC = mm_half(t, 2)
                        halves.append(pmC)
                        window_stt(mw, 0, 128, pmB, wl - HALF, t)
                        window_stt(mw, 128, 128, pmC, 0, t)
                    elif wl + W <= HALF:
                        window_stt(mw, 0, W, pmA, wl, t)
                    elif wl >= HALF:
                        window_stt(mw, 0, W, pmB, wl - HALF, t)
                    else:  # t == 7: [896, 1152) spans both halves
                        window_stt(mw, 0, HALF - wl, pmA, wl, t)
                        window_stt(mw, HALF - wl, wl + W - HALF, pmB, 0, t)
                    nc.vector.tensor_reduce(
                        mdm_all[:, t:t + 1], mw[:],
                        axis=mybir.AxisListType.X, op=Alu.max,
                    )
                    if t < NDVE:
                        nc.vector.tensor_scalar(
                            dmq[:, t:t + 1], mdm_all[:, t:t + 1], -1.0, None,
                            op0=Alu.mult,
                        )
                    for i, pm in enumerate(halves):
                        sign_evac(pm, t, i)

                nc.vector.tensor_tensor(
                    out=acc[:], in0=sgn_all[:, 0:NSLABS],
                    in1=sgn_all[:, NSLABS:2 * NSLABS], op=Alu.add,
                )

                # ---- remaining column blocks: matmul + Sign evac only;
                # fold each block's sign-sums into acc as it completes ----
                for sub in range(2, NSUB):
                    for t in range(NSLABS):
                        if sub == 2 and t == NSLABS - 1:
                            continue  # done in phase 0
                        pm = mm_half(t, sub)
                        sign_evac(pm, t, sub)
                    nc.vector.tensor_tensor(
                        out=acc[:], in0=acc[:],
                        in1=sgn_all[:, NSLABS * sub:NSLABS * (sub + 1)],
                        op=Alu.add,
                    )

                # ---- match = (cnt <= k) / (ssum >= G-1-2k) ----
                match16 = small.tile([128, NSLABS], f32, tag="match")
                # DVE-evacuated slabs hold counts: match <=> cnt <= k;
                # ACT slabs hold sign-sums: match <=> ssum >= G-1-2k.
                nc.vector.tensor_scalar(
                    match16[:, 0:NDVE], acc[:, 0:NDVE], float(k), None,
                    op0=Alu.is_le,
                )
                nc.vector.tensor_scalar(
                    match16[:, NDVE:], acc[:, NDVE:], thresh, None,
                    op0=Alu.is_ge,
                )
                msum = small.tile([128, 1], f32, tag="msum")
                nc.vector.reduce_sum(
                    msum[:], match16[:], axis=mybir.AxisListType.X
                )

            with tc.tile_pool(name="fin", bufs=1, space="PSUM") as finp:
                pf = finp.tile([1, 1], f32)
                nc.tensor.matmul(pf[:], ones_col[:], msum[:], start=True, stop=True)
                osb = singles.tile([1, 1], f32)
                nc.scalar.activation(osb[:], pf[:], Act.Copy)
                nc.sync.dma_start(out_d[:], osb[:])

    nc.finalize()
    return nc


def _prepare_inputs(embeddings, labels):
    """Sort each flank by label, build per-core rotated fp16 split inputs."""
    emb = np.ascontiguousarray(np.asarray(embeddings, dtype=np.float32))
    lab = np.asarray(labels)
    in_maps = []
    for f in range(NUM_FLANKS):
        ef = emb[f * G:(f + 1) * G]
        lf = lab[f * G:(f + 1) * G]
        order = np.argsort(lf, kind="stable")
        ef, lf = ef[order], lf[order]
        # window-margin safety: same-label runs must fit in M
        runs = np.diff(
            np.flatnonzero(np.concatenate(([True], lf[1:] != lf[:-1], [True])))
        )
        assert runs.max() <= M, f"label run {runs.max()} exceeds window margin {M}"
        for cc in range(CORES_PER_FLANK):
            r = Q * cc
            # db col j = sorted row (j + r - ROLL) mod G ; query i = col i+ROLL
            db = np.ascontiguousarray(np.roll(ef, ROLL - r, axis=0))
            labdb = np.roll(lf, ROLL - r).astype(np.float32)
            h = db.astype(np.float16)
            low = (db - h.astype(np.float32)).astype(np.float16)
            hT = np.ascontiguousarray(h.T)           # [256, G]
            lT = np.ascontiguousarray(low.T)
            sqb = np.einsum(
                "ij,ij->i", db.astype(np.float64), db.astype(np.float64)
            ).astype(np.float32)
            sh = sqb.astype(np.float16)
            slo = (sqb - sh.astype(np.float32)).astype(np.float16)
            l0 = lT[0:128].copy()
            l0[0, :] = sh                            # +sq rides rows 0,1
            l0[1, :] = slo
            qs = slice(ROLL, ROLL + Q)
            hq0 = np.ascontiguousarray(-2.0 * hT[0:128, qs]).astype(np.float16)
            hq1 = np.ascontiguousarray(-2.0 * hT[128:256, qs]).astype(np.float16)
            hqm = hq0.copy()
            hqm[0:2, :] = np.float16(1.0)
            # negative label-window mask, [128, 16*256]
            nen = np.empty((128, NSLABS * W), dtype=np.float32)
            for t in range(NSLABS):
                winl = labdb[128 * t:128 * t + W]       # window labels
                ql = labdb[128 * t + ROLL:128 * t + ROLL + 128]  # query labels
                ne = BIG * (winl[None, :] != ql[:, None]).astype(np.float32)
                ne[np.arange(128), np.arange(128) + ROLL] += BIG  # self
                nen[:, W * t:W * (t + 1)] = -ne
            in_maps.append({
                "h0": np.ascontiguousarray(hT[0:128]),
                "h1": np.ascontiguousarray(hT[128:256]),
                "l0": l0,
                "hq0": hq0,
                "hqm": hqm,
                "hq1": hq1,
                "nen": nen,
            })
    return in_maps


def kernel(embeddings, labels, flanks, k):
    from concourse.bass_utils import run_bass_kernel_spmd

    k = int(k)
    if ("nc", k) not in _cached:
        _cached[("nc", k)] = _build_program(k)
    nc = _cached[("nc", k)]
    in_maps = _prepare_inputs(embeddings, labels)
    res = run_bass_kernel_spmd(nc, in_maps, list(range(NCORES)))
    total = sum(float(r["out"][0, 0]) for r in res.results)
    return np.float32(total / N)


if __name__ == "__main__":
    sys.path.insert(0, os.path.dirname(os.path.abspath(__file__)))
    from reference import setup_inputs, reference

    inputs = setup_inputs()
    expected = float(reference(**inputs))
    got = float(kernel(**{kk: np.asarray(v) for kk, v in inputs.items()}))
    rel = abs(got - expected) / abs(got) if got else 1.0
    print(f"expected={expected} got={got} rel={rel:.3e}")
